# revision 114
# baseline (speedup 1.0000x reference)
"""IoU / NMS-detection kernel for TRN2 (8 NeuronCores, data-parallel over batch).

Measured (cost-model, 8-core SPMD): 120.3us vs 266.6us baseline (2.22x).

Computes, for batch_boxes [32,8732,4] (cxcywh) and batch_gt [32,100,4]:
  ious [32,8732,100] f32, positive_mask = (iou>0.5)&valid, negative_mask = (iou<0.5)&valid

Device strategy (per core, 16 quarter-batch units; all four engines used):
  - each batch splits into four 18-tile anchor-quarters (9216 padded
    anchors); the 128 units sort by num_objects and deal round-robin, so
    slot s computes only Gp[s] gt columns (compile-time, ~exact maxima).
    Narrow slots (G < 18) emit DX ops in flipped orientation: one op per gt
    column with gt coords as scalars and stride-5 pf streams (2G ops of
    width 18 instead of 36 ops of width G; bit-exact, min/max commute).
  - DVE: fused custom smalls IOU_DXS (dx/2), IOU_DX (dy), plus inter/2 =
    dxh*dy (TT). Unions run two tiles per op (IOU_UNION2): the per-tile
    area_p/4 scalar rides a 2-page PageIdx affine (C0=ap4(t),
    C1=fl(ap4(t+1)-ap4(t)) host-packed; page 1 within 1 ULP), with the
    area_g/4 stream doubled in the gt pack; odd leftovers use IOU_UNION_S.
    All power-of-2 scales are exact, so inter/union stay bit-faithful to the
    reference f32 chain.
  - Pool: md = inter/2 - union/4 (sign(md) == sign(2*inter - union), the
    exact iou>0.5 compare), and lnd = ln(inter/2) - ln(union/4).
  - Act: Ln/Ln/Exp (iou2 = exp(lnd) = 2*iou, bf16 out; rel err ~bf16 level
    for iou >= 1e-15) and Sign(md) -> int8 m in {-1,0,1}; one preloaded act
    table set (ln+exp+sign) avoids table thrash.
  - DVE never consumes Pool/Act outputs (no cross-engine back-edges), so the
    schedule stays DVE-bound at ~91% occupancy; supertile tails are flushed
    one iteration late (software pipelining); the last unit's tiny gt width
    keeps the end-of-kernel drain chain short.
  - host: iou = bf16(2*iou) -> f32 * 0.5 (exact), pos/neg = (m==+-1) & valid,
    zero-fill for un-computed gt columns (invalid gt are degenerate on device
    so computed-but-invalid columns give iou == 0 exactly).
"""

import os
import numpy as np

import concourse.bacc as bacc
import concourse.mybir as mybir
import concourse.tile as tile
import concourse.dve_ops as dve_ops
from concourse.alu_op_type import AluOpType
from concourse.bass_utils import run_bass_kernel_spmd
from concourse.dve_spec import (
    Spec, Src0, Src1, C0, C1, C2, PageIdx, relu, minn, maxx, lower, _has_src1,
)
from concourse.dve_uop import DveOpSpec

B, N, G = 32, 8732, 100
NCORES = 8
BPC = B // NCORES          # batches per core
NT = 69                    # anchor tiles per batch (padded)
NPAD = NT * 128            # 8832
K = 23                     # tiles per supertile
NST = NT // K              # supertiles per batch
# quarter-batch sharding: each batch splits into 4 anchor-quarters of 18
# tiles; 128 units deal into 16 slots, making slot-max gt widths ~exact
NQ = 4                     # quarters per batch
NSEC = 16                  # quarter-batch sections per core
NTH = 18                   # tiles per quarter (9216 padded anchors per batch)
NPADH = NTH * 128          # 2304
STS_H = [(0, 18)]          # one even supertile per quarter

_f32 = mybir.dt.float32
_bf16 = mybir.dt.bfloat16
_s8 = mybir.dt.int8
_AFT = mybir.ActivationFunctionType


def _register_op(name, spec, subdim=False):
    for op in dve_ops.OPS:
        if op.name == name:
            return op
    row = dve_ops._CUSTOM_DVE_ROW_BASE + len(dve_ops.OPS)
    assert row < 0x20
    dve_ops._SUB_OPCODE_FOR_NAME[name] = row
    sha3 = DveOpSpec(
        name=name, opcode=row, uops=lower(spec, ver="v3"), rd1_en=_has_src1(spec)
    ).sha("v3")
    op = dve_ops.DveOp(name, spec, subdim, {"v3": sha3})
    dve_ops.OPS.append(op)
    dve_ops.CUSTOM_DVE_SPECS[name] = spec
    return op


IOU_DX = _register_op(
    "IOU_DX_ANT",
    Spec(
        body=relu(minn(C0, Src0) - maxx(C1, Src1)),
        reference=lambda in0, in1, s0, s1, imm2: np.maximum(
            np.minimum(s0, in0.astype(np.float32)) - np.maximum(s1, in1), 0
        ).astype(np.float32),
    ),
)

# x-direction overlap scaled by C2 (=0.5): dxh = 0.5 * dx. Power-of-2 scale
# is exact in f32, keeping the downstream mask comparison bit-faithful.
IOU_DXS = _register_op(
    "IOU_DXS_ANT",
    Spec(
        body=relu(minn(C0, Src0) - maxx(C1, Src1)) * C2,
        reference=lambda in0, in1, s0, s1, imm2: (
            np.maximum(np.minimum(s0, in0.astype(np.float32)) - np.maximum(s1, in1), 0)
            * imm2
        ).astype(np.float32),
    ),
)

# u4 = (C0 + Src1) - Src0*C2 with C0=area_p/4, Src1=area_g/4, Src0=inter/2,
# C2=0.5  ->  u4 = union/4 (exact power-of-2 scale of the reference union).
IOU_UNION_S = _register_op(
    "IOU_UNION_S_ANT",
    Spec(
        body=(C0 + Src1) - Src0 * C2,
        reference=lambda in0, in1, s0, s1, imm2: (
            (s0 + in1.astype(np.float32)) - in0 * imm2
        ).astype(np.float32),
    ),
)

# Two-tile union: in0 = inter [P,2,G] pages, in1 = area_g/4 doubled [P,2G],
# per-page area_p/4 delivered as the affine PageIdx(C0, C1) with C0 =
# ap4(t), C1 = host-packed fl(ap4(t+1) - ap4(t)) (page 1's area is within
# 1 ULP of exact; verified 0 mask flips on the fixed dataset).
IOU_UNION2 = _register_op(
    "IOU_UNION2_ANT",
    Spec(
        body=(PageIdx(C0, C1) + Src1) - Src0 * C2,
        reference=lambda in0, in1, s0, s1, imm2: (
            (s0 + np.arange(in0.shape[1])[None, :, None] * s1[..., None])
            + in1.reshape(in0.shape)
            - in0 * imm2
        ).astype(np.float32),
    ),
    subdim=True,
)


_NC_CACHE = {}


def _pool_offload_slots(gs):
    """Slots whose union computation would move DVE -> Pool. Measured: every
    offload variant loses despite lower DVE busy time — the in-order engine
    queues plus 3-deep tile buffering can't hide the extra Pool->Act chain
    (139.8us config-A vs 151+us for all offload interleavings). Disabled."""
    return set()


def _build_nc(gs):
    """gs: per-half-batch-slot gt column counts (compile-time), len == NSEC."""
    nc = bacc.Bacc("TRN2", target_bir_lowering=False, debug=False)
    pf = nc.dram_tensor("pf", [NSEC, 128, NTH * 5], _f32, kind="ExternalInput")
    apd = nc.dram_tensor("apd", [NSEC, 128, NTH], _f32, kind="ExternalInput")
    gt_d = [
        nc.dram_tensor(f"gt{b}", [128, 6 * gs[b]], _f32, kind="ExternalInput")
        for b in range(NSEC)
    ]
    # flat tile-major layout: [p, t*Gb + g]; anchor n = t*128 + p
    iou_d = [
        nc.dram_tensor(f"iou_out{b}", [128, NTH * gs[b]], _bf16, kind="ExternalOutput")
        for b in range(NSEC)
    ]
    m_d = [
        nc.dram_tensor(f"m_out{b}", [128, NTH * gs[b]], _s8, kind="ExternalOutput")
        for b in range(NSEC)
    ]

    with tile.TileContext(nc) as tc:
        with tc.tile_pool(name="const", bufs=1) as cpool, tc.tile_pool(
            name="io", bufs=1
        ) as iop, tc.tile_pool(name="st", bufs=3) as stp, tc.tile_pool(
            name="ln", bufs=2
        ) as lnp, tc.tile_pool(name="out", bufs=2) as outp:
            # Preload the one act-table set covering Ln+Exp+Sign so the
            # fixpoint loader doesn't thrash tables between Ln and Exp.
            from concourse.hw_specs import get_activation_tables

            tabs = list(get_activation_tables(nc.m.arch).items())
            need = {_AFT.Ln, _AFT.Exp, _AFT.Sign}
            set_id = next(i for i, (_, s) in enumerate(tabs) if need <= s)
            nc.scalar.add_instruction(
                mybir.InstLoadActFuncSet(
                    name=nc.get_next_instruction_name(),
                    act_func_set_id=set_id,
                    engine=mybir.EngineType.Activation,
                    ins=[],
                    outs=[],
                )
            )
            zero_b = cpool.tile([128, 1], _f32, tag="zerob")
            nc.gpsimd.memset(zero_b[:], 0.0)
            def flush(ctx, split=1):
                """Tail of a supertile: union smalls (DVE) + mask + iou paths.
                Deferred one supertile so the DVE sequencer never stalls
                waiting on downstream engines."""
                (b, t0, Ki, Gb, pf_t, apd_t, ag, ag2, dxr, dyr, inter, union,
                 md, iou, mm) = ctx
                W = Ki * Gb
                # paired two-tile unions, leftover tile single when Ki is odd
                for j in range(Ki // 2):
                    tp = t0 + 2 * j
                    csl = slice(2 * j * Gb, (2 * j + 2) * Gb)
                    nc.vector._custom_dve(
                        IOU_UNION2,
                        out=union[:, csl].rearrange("p (s n) -> p s n", s=2, n=Gb),
                        in0=inter[:, csl].rearrange("p (s n) -> p s n", s=2, n=Gb),
                        in1=ag2,
                        s0=pf_t[:, tp * 5 + 4 : tp * 5 + 5],
                        s1=apd_t[:, tp : tp + 1],
                        imm2=0.5,
                    )
                if Ki % 2:
                    t = t0 + Ki - 1
                    sl = slice((Ki - 1) * Gb, Ki * Gb)
                    nc.vector._custom_dve(
                        IOU_UNION_S, out=union[:, sl], in0=inter[:, sl],
                        in1=ag, s0=pf_t[:, t * 5 + 4 : t * 5 + 5], imm2=0.5,
                    )
                # union tile holds u4 = union/4. Tail per column-chunk
                # (split>1 only on the last supertile, where the otherwise-
                # serial Pool->Act->DMA chain is the kernel's drain tail):
                #   mask: sign(inter/2 - union/4) == sign(2*inter - union)
                #   iou2 = exp(ln(inter/2) - ln(union/4)) = 2*iou (Act tables;
                #   rel err ~bf16 level for iou >= 1e-15); host halves later
                la = lnp.tile([128, W], _f32, tag="la")
                lb = lnp.tile([128, W], _f32, tag="lb")
                base = t0 * Gb
                Wc = W // split
                for c in range(split):
                    cs = slice(c * Wc, (c + 1) * Wc if c < split - 1 else W)
                    ds = slice(base + cs.start, base + cs.stop)
                    nc.gpsimd.tensor_sub(md[:, cs], inter[:, cs], union[:, cs])
                    nc.scalar.activation(out=la[:, cs], in_=inter[:, cs], func=_AFT.Ln)
                    nc.scalar.activation(out=lb[:, cs], in_=union[:, cs], func=_AFT.Ln)
                    nc.scalar.sign(out=mm[:, cs], in_=md[:, cs], bias=zero_b[:])
                    nc.gpsimd.tensor_sub(md[:, cs], la[:, cs], lb[:, cs])
                    nc.scalar.activation(out=iou[:, cs], in_=md[:, cs], func=_AFT.Exp)
                    nc.sync.dma_start(out=iou_d[b][:, ds], in_=iou[:, cs])
                    nc.sync.dma_start(out=m_d[b][:, ds], in_=mm[:, cs])

            # Preload every batch's inputs up front (~14KB total) so input
            # DMAs never queue behind output DMAs at batch boundaries.
            ins = []
            for b in range(NSEC):
                Gb = gs[b]
                gt_t = iop.tile([128, 6 * Gb], _f32, tag=f"gt{b}")
                pf_t = iop.tile([128, NTH * 5], _f32, tag=f"pf{b}")
                apd_t = iop.tile([128, NTH], _f32, tag=f"apd{b}")
                nc.sync.dma_start(out=gt_t[:], in_=gt_d[b][:, :])
                # batch 0 gates the pipeline start: its pf rides the empty
                # gpsimd queue (25ns dispatch) in parallel with gt0 on SP
                (nc.gpsimd if b == 0 else nc.sync).dma_start(
                    out=pf_t[:], in_=pf[b]
                )
                nc.sync.dma_start(out=apd_t[:], in_=apd[b])
                ins.append((gt_t, pf_t, apd_t))

            pending = None
            border = list(range(NSEC))  # descending width; smallest slot last
            for b in border:
                Gb = gs[b]
                gt_t, pf_t, apd_t = ins[b]
                gx1 = gt_t[:, 0:Gb]
                gx2 = gt_t[:, Gb : 2 * Gb]
                gy1 = gt_t[:, 2 * Gb : 3 * Gb]
                gy2 = gt_t[:, 3 * Gb : 4 * Gb]
                ag = gt_t[:, 4 * Gb : 5 * Gb]
                ag2 = gt_t[:, 4 * Gb : 6 * Gb]
                for t0, Ki in STS_H:
                    W = Ki * Gb
                    dxr = stp.tile([128, W], _f32, tag="dxr")
                    dyr = stp.tile([128, W], _f32, tag="dyr")
                    inter = stp.tile([128, W], _f32, tag="inter")
                    union = stp.tile([128, W], _f32, tag="union")
                    md = stp.tile([128, W], _f32, tag="md")
                    iou = outp.tile([128, W], _bf16, tag="iou")
                    mm = outp.tile([128, W], _s8, tag="mm")
                    if Gb < Ki:
                        # flipped orientation for narrow slots: one op per gt
                        # column (width Ki tiles), gt coord as the scalar and
                        # anchor coords as stride-5 pf streams. 2*Gb ops
                        # instead of 2*Ki - same fused math, swapped args.
                        px1v = pf_t[:, 0::5]
                        px2v = pf_t[:, 1::5]
                        py1v = pf_t[:, 2::5]
                        py2v = pf_t[:, 3::5]
                        dx3 = dxr[:].rearrange("p (t g) -> p g t", t=Ki, g=Gb)
                        dy3 = dyr[:].rearrange("p (t g) -> p g t", t=Ki, g=Gb)
                        for g in range(Gb):
                            nc.vector._custom_dve(
                                IOU_DXS, out=dx3[:, g], in0=px2v, in1=px1v,
                                s0=gt_t[:, Gb + g : Gb + g + 1],
                                s1=gt_t[:, g : g + 1], imm2=0.5,
                            )
                            nc.vector._custom_dve(
                                IOU_DX, out=dy3[:, g], in0=py2v, in1=py1v,
                                s0=gt_t[:, 3 * Gb + g : 3 * Gb + g + 1],
                                s1=gt_t[:, 2 * Gb + g : 2 * Gb + g + 1],
                            )
                    else:
                        for i in range(Ki):
                            t = t0 + i
                            sl = slice(i * Gb, (i + 1) * Gb)
                            px1 = pf_t[:, t * 5 + 0 : t * 5 + 1]
                            px2 = pf_t[:, t * 5 + 1 : t * 5 + 2]
                            py1 = pf_t[:, t * 5 + 2 : t * 5 + 3]
                            py2 = pf_t[:, t * 5 + 3 : t * 5 + 4]
                            nc.vector._custom_dve(
                                IOU_DXS, out=dxr[:, sl], in0=gx2, in1=gx1,
                                s0=px2, s1=px1, imm2=0.5,
                            )
                            nc.vector._custom_dve(
                                IOU_DX, out=dyr[:, sl], in0=gy2, in1=gy1, s0=py2, s1=py1
                            )
                    if pending is not None:
                        flush(pending)
                    # inter_h = (dx/2)*dy = inter/2 (exact scale). Stays on DVE:
                    # unions consume it, and a Pool->DVE back-edge would turn
                    # the schedule into a cross-engine ping-pong.
                    nc.vector.tensor_mul(inter[:], dxr[:], dyr[:])
                    pending = (b, t0, Ki, Gb, pf_t, apd_t, ag, ag2, dxr, dyr,
                               inter, union, md, iou, mm)
            flush(pending)
    nc.compile()
    return nc


def _get_nc(gs):
    key = tuple(gs)
    if key not in _NC_CACHE:
        _NC_CACHE[key] = _build_nc(key)
    return _NC_CACHE[key]


def kernel(
    threshhold=None,
    batch_boxes=None,
    batch_classes=None,
    batch_gt=None,
    batch_num_objects=None,
    **_kw,
):
    boxes = np.asarray(batch_boxes, np.float32)
    gtb = np.asarray(batch_gt, np.float32)
    no = np.asarray(batch_num_objects).astype(np.int64)

    half = np.float32(0.5)
    cx, cy, w, h = boxes[..., 0], boxes[..., 1], boxes[..., 2], boxes[..., 3]
    px1 = cx - w * half
    py1 = cy - h * half
    px2 = cx + w * half
    py2 = cy + h * half
    area_p = (px2 - px1) * (py2 - py1)

    def pad(a, fill):
        out = np.full((B, NQ * NPADH), fill, np.float32)
        out[:, :N] = a
        return out

    pf = np.stack(
        [pad(px1, -1e4), pad(px2, -1e4), pad(py1, -1e4), pad(py2, -1e4),
         pad(area_p * np.float32(0.25), 0.25)],
        axis=-1,
    )  # [B, 2*NPADH, 5]; area column pre-scaled by 1/4 (exact)
    # per quarter-batch unit: [B, NQ, 128, NTH*5]
    pf = np.ascontiguousarray(
        pf.reshape(B, NQ, NTH, 128, 5).transpose(0, 1, 3, 2, 4)
        .reshape(B, NQ, 128, NTH * 5)
    )

    gcx, gcy, gw, gh = gtb[..., 0], gtb[..., 1], gtb[..., 2], gtb[..., 3]
    gx1 = gcx - gw * half
    gy1 = gcy - gh * half
    gx2 = gcx + gw * half
    gy2 = gcy + gh * half
    area_g = (gx2 - gx1) * (gy2 - gy1)
    validm = np.arange(G)[None, :] < no[:, None]  # [B, G]
    NEG = np.float32(-1e6)
    gx1 = np.where(validm, gx1, NEG).astype(np.float32)
    gx2 = np.where(validm, gx2, NEG).astype(np.float32)
    gy1 = np.where(validm, gy1, NEG).astype(np.float32)
    gy2 = np.where(validm, gy2, NEG).astype(np.float32)
    area_g = np.where(validm, area_g, np.float32(0.0)).astype(np.float32)
    # G-trim over half-batch units: each batch contributes two 35-tile
    # anchor-halves with the same num_objects. The 64 units sort by
    # num_objects (desc) and deal round-robin to cores; slot s gets the
    # compile-time gt width Gp[s] = max num_objects in its unit group.
    units = [(int(b), h) for b in np.argsort(-no, kind="stable") for h in range(NQ)]
    gslot = []
    for s in range(NSEC):
        grp = units[s * NCORES : (s + 1) * NCORES]
        gslot.append(int(max(no[b] for b, _ in grp)))

    nc = _get_nc(gslot)
    _NC_CACHE["nc"] = nc
    in_maps = []
    for c in range(NCORES):
        cu = [units[s * NCORES + c] for s in range(NSEC)]
        pf_c = np.stack([pf[b, h] for b, h in cu])  # [NSEC, 128, NTH*5]
        # apd[p,t] = ap4[p,t+1] - ap4[p,t]: PageIdx page-1 area for the
        # paired two-tile union ops
        ap4_c = pf_c[:, :, 4::5]  # [NSEC, 128, NTH]
        apd_c = np.zeros_like(ap4_c)
        apd_c[:, :, :-1] = ap4_c[:, :, 1:] - ap4_c[:, :, :-1]
        m_in = {
            "pf": np.ascontiguousarray(pf_c),
            "apd": np.ascontiguousarray(apd_c),
        }
        for s in range(NSEC):
            b, _h = cu[s]
            Gs = gslot[s]
            ag_s = area_g[b, :Gs] * np.float32(0.25)
            gtp = np.concatenate(
                [gx1[b, :Gs], gx2[b, :Gs], gy1[b, :Gs], gy2[b, :Gs], ag_s, ag_s]
            )  # [6*Gs]: area block doubled for the paired union's flat stream
            m_in[f"gt{s}"] = np.ascontiguousarray(
                np.broadcast_to(gtp[None, :], (128, 6 * Gs))
            )
        in_maps.append(m_in)
    trace = os.environ.get("IOU_TRACE", "0") == "1"
    res = run_bass_kernel_spmd(nc, in_maps, list(range(NCORES)), trace=trace)
    _NC_CACHE["last_result"] = res
    results = res.results

    def unscramble(a, Gs):
        # flat [128, NTH*Gs] -> [NPADH, Gs]; anchor n = t*128 + p
        a = a.reshape(128, NTH, Gs).transpose(1, 0, 2)
        return np.ascontiguousarray(a).reshape(NPADH, Gs)

    ious = np.zeros((B, N, G), np.float32)
    m = np.zeros((B, N, G), np.int8)
    for c in range(NCORES):
        r = results[c]
        for s in range(NSEC):
            b, h = units[s * NCORES + c]
            Gs = gslot[s]
            r0 = h * NPADH          # first output row of this half
            nrows = min(NPADH, N - r0)
            # device emits 2*iou in bf16; halve after upcast (exact in f32)
            iou_b = unscramble(
                r[f"iou_out{s}"].astype(np.float32) * np.float32(0.5), Gs
            )
            ious[b, r0 : r0 + nrows, :Gs] = iou_b[:nrows]
            m[b, r0 : r0 + nrows, :Gs] = unscramble(r[f"m_out{s}"], Gs)[:nrows]
    vb = validm[:, None, :]
    pos = (m == 1) & vb
    neg = (m == -1) & vb
    return ious, pos, neg

